# revision 8
# baseline (speedup 1.0000x reference)
"""Mamba-1 selective SSM block on 8 trn2 NeuronCores.

Sharding: 2 batch-groups x 4 channel-shards. Core c handles batch c//4 and
d_inner channels [(c%4)*512, (c%4+1)*512). Cross-core comm: bf16 AllReduce of
the x_proj partial [96, 512] per L-half within each 4-core batch group. Host
sums the 4 partial out_proj outputs per batch.

The network is software-pipelined over two L-halves so the AllReduce hides
under compute of the other half. State tensors live as tiles of
[128 partitions, 8*LH] where partition p = n*8 + d_sub covers 8 channels x 16
states and the free dim concatenates 8 groups' L-half segments; one DVE scan
per (q, hf, half) covers 8 groups using a=0 segment resets (the carry across
halves is folded into the first bu element of each segment). delta replicates
onto (n,d) via PE selector matmuls + Act exp; du replicates via DRAM DMA
issued from the idle gpsimd queue. Selector matmuls run j-outer over channel
pairs so LDWEIGHTS is shared.
"""

import numpy as np
import ml_dtypes

import concourse.bacc as bacc
import concourse.mybir as mybir
import concourse.tile as tile
from concourse import bass_utils

BF16 = mybir.dt.bfloat16
F32 = mybir.dt.float32
AF = mybir.ActivationFunctionType
OP = mybir.AluOpType

L = 1024          # sequence length
DM = 1024         # model dim
DL = 512          # local d_inner channels per core
NQ = 4            # channel chunks of 128 per core
NGRP = 64         # DL/8 groups per core
RANK = 64         # dt_rank
LH = 512          # L half

_CACHE = {}


def _build(sim=False):
    nc = bacc.Bacc("TRN2", target_bir_lowering=False, debug=False, num_devices=8)

    xT = nc.dram_tensor("xT", [DM, L], BF16, kind="ExternalInput")
    w_in = nc.dram_tensor("w_in", [DM, 2 * DL], BF16, kind="ExternalInput")
    w_xp = nc.dram_tensor("w_xp", [DL, 96], BF16, kind="ExternalInput")
    w_dt = nc.dram_tensor("w_dt", [RANK, DL], BF16, kind="ExternalInput")
    dt_b = nc.dram_tensor("dt_b", [128, NQ], F32, kind="ExternalInput")
    w_out = nc.dram_tensor("w_out", [DL, DM], BF16, kind="ExternalInput")
    a_cols = nc.dram_tensor("a_cols", [128, NGRP], F32, kind="ExternalInput")
    d_col = nc.dram_tensor("d_col", [128, NQ], F32, kind="ExternalInput")
    convd = nc.dram_tensor("convd", [128, NQ * 4 * 128], BF16, kind="ExternalInput")
    convb = nc.dram_tensor("convb", [128, NQ], F32, kind="ExternalInput")
    selr = nc.dram_tensor("selr", [128, 16 * 128], BF16, kind="ExternalInput")
    selo = nc.dram_tensor("selo", [128, 16 * 128], BF16, kind="ExternalInput")
    selbc = nc.dram_tensor("selbc", [32, 2 * 128], BF16, kind="ExternalInput")
    out = nc.dram_tensor("out", [DM, L], BF16, kind="ExternalOutput")

    with tile.TileContext(nc) as tc:
        with (
            tc.tile_pool(name="const", bufs=1) as cp,
            tc.tile_pool(name="acts", bufs=1) as ap,
            tc.tile_pool(name="wpool", bufs=1) as wp,
            tc.tile_pool(name="dram", bufs=1, space="DRAM") as dp,
            tc.tile_pool(name="grp", bufs=4) as gp,
            tc.tile_pool(name="small", bufs=2) as sp2,
            tc.tile_pool(name="durep", bufs=6) as drp,
            tc.tile_pool(name="sa", bufs=2) as sa,
            tc.tile_pool(name="ps_small", bufs=2, space="PSUM") as pss,
            tc.tile_pool(name="ps_rep", bufs=4, space="PSUM") as psr,
            tc.tile_pool(name="ps_y", bufs=2, space="PSUM") as psy,
        ):
            # ---- persistent weights (critical-path order) ----
            xT_sb = wp.tile([128, 8 * L], BF16, tag="xT")
            nc.sync.dma_start(xT_sb[:].rearrange("p (k l) -> p k l", k=8),
                              xT.ap().rearrange("(k p) l -> p k l", p=128))
            w_in_sb = wp.tile([128, 8 * 1024], BF16, tag="w_in")
            nc.sync.dma_start(w_in_sb[:].rearrange("p (k m) -> p k m", k=8),
                              w_in.ap().rearrange("(k p) m -> p k m", p=128))
            convd_sb = cp.tile([128, NQ * 4 * 128], BF16, tag="convd")
            nc.sync.dma_start(convd_sb[:], convd.ap())
            convb_sb = cp.tile([128, NQ], F32, tag="convb")
            nc.sync.dma_start(convb_sb[:], convb.ap())
            w_xp_sb = wp.tile([128, 4 * 96], BF16, tag="w_xp")
            nc.sync.dma_start(w_xp_sb[:].rearrange("p (k m) -> p k m", k=4),
                              w_xp.ap().rearrange("(k p) m -> p k m", p=128))
            w_dt_sb = wp.tile([64, DL], BF16, tag="w_dt")
            nc.sync.dma_start(w_dt_sb[:], w_dt.ap())
            selr_sb = cp.tile([128, 16 * 128], BF16, tag="selr")
            nc.sync.dma_start(selr_sb[:], selr.ap())
            selo_sb = cp.tile([128, 16 * 128], BF16, tag="selo")
            nc.sync.dma_start(selo_sb[:], selo.ap())
            selbc_sb = cp.tile([32, 2 * 128], BF16, tag="selbc")
            nc.sync.dma_start(selbc_sb[:], selbc.ap())
            dtb_sb = cp.tile([128, NQ], F32, tag="dtb")
            nc.sync.dma_start(dtb_sb[:], dt_b.ap())
            acol_sb = cp.tile([128, NGRP], F32, tag="acol")
            nc.sync.dma_start(acol_sb[:], a_cols.ap())
            dcol_sb = cp.tile([128, NQ], F32, tag="dcol")
            nc.sync.dma_start(dcol_sb[:], d_col.ap())
            w_out_sb = wp.tile([128, 4 * 1024], BF16, tag="w_out")
            nc.sync.dma_start(w_out_sb[:].rearrange("p (k m) -> p k m", k=4),
                              w_out.ap().rearrange("(k p) m -> p k m", p=128))

            # ---- persistent activations ----
            xin = [ap.tile([128, 3 + L], BF16, tag=f"xin{q}", name=f"xin{q}") for q in range(NQ)]
            silu_z = [ap.tile([128, L], BF16, tag=f"sz{q}", name=f"sz{q}") for q in range(NQ)]
            u = [ap.tile([128, L], BF16, tag=f"u{q}", name=f"u{q}") for q in range(NQ)]
            delta = [ap.tile([128, L], BF16, tag=f"delta{q}", name=f"delta{q}") for q in range(NQ)]
            du = [ap.tile([128, L], BF16, tag=f"du{q}", name=f"du{q}") for q in range(NQ)]
            ygate = [ap.tile([128, L], BF16, tag=f"yg{q}", name=f"yg{q}") for q in range(NQ)]
            b_rep = ap.tile([128, L], BF16, tag="brep")
            c_rep = ap.tile([128, L], BF16, tag="crep")
            carry = [ap.tile([128, 8], BF16, tag=f"carry{i}", name=f"carry{i}")
                     for i in range(2 * NQ)]
            du_d = [dp.tile([128, L], BF16, name=f"du_d{q}") for q in range(NQ)]
            cc_in = [dp.tile([96, LH], BF16, name=f"cc_in{h}") for h in range(2)]
            cc_out = [dp.tile([96, LH], BF16, name=f"cc_out{h}") for h in range(2)]

            for q in range(NQ):
                nc.vector.memset(xin[q][:, 0:3], 0.0)

            def hs(h):
                return slice(h * LH, (h + 1) * LH)

            def xs(h):
                return slice(3 + h * LH, 3 + (h + 1) * LH)

            # ================= phases 1-3 for one half =================
            def phases123(h):
                for q in range(NQ):
                    ps = pss.tile([128, LH], F32, tag="ps")
                    for k in range(8):
                        nc.tensor.matmul(
                            ps[:],
                            w_in_sb[:, q * 128 + k * 1024:(q + 1) * 128 + k * 1024],
                            xT_sb[:, k * L + h * LH:k * L + h * LH + LH],
                            start=(k == 0), stop=(k == 7))
                    nc.scalar.copy(xin[q][:, xs(h)], ps[:])
                    psc = pss.tile([128, LH], F32, tag="ps")
                    for k in range(4):
                        nc.tensor.matmul(
                            psc[:],
                            convd_sb[:, (q * 4 + k) * 128:(q * 4 + k + 1) * 128],
                            xin[q][:, k + h * LH:k + h * LH + LH],
                            start=(k == 0), stop=(k == 3))
                    nc.scalar.activation(u[q][:, hs(h)], psc[:], AF.Silu,
                                         bias=convb_sb[:, q:q + 1])
                # x_proj partial -> DRAM (bf16) -> AllReduce
                psx = pss.tile([96, LH], F32, tag="ps")
                for q in range(NQ):
                    nc.tensor.matmul(psx[:], w_xp_sb[:, q * 96:(q + 1) * 96],
                                     u[q][:, hs(h)], start=(q == 0), stop=(q == 3))
                xpc = sa.tile([96, LH], BF16, tag="xpc")
                nc.scalar.copy(xpc[:], psx[:])
                nc.sync.dma_start(cc_in[h][:], xpc[:])
                if sim:
                    nc.sync.dma_start(cc_out[h][:], cc_in[h][:])
                else:
                    nc.gpsimd.collective_compute(
                        "AllReduce", OP.add,
                        replica_groups=[[0, 1, 2, 3], [4, 5, 6, 7]],
                        ins=[cc_in[h].opt()], outs=[cc_out[h].opt()])
                # z half (not needed until ygate; runs during AR)
                for m in range(4, 8):
                    ps = pss.tile([128, LH], F32, tag="ps")
                    for k in range(8):
                        nc.tensor.matmul(
                            ps[:],
                            w_in_sb[:, m * 128 + k * 1024:(m + 1) * 128 + k * 1024],
                            xT_sb[:, k * L + h * LH:k * L + h * LH + LH],
                            start=(k == 0), stop=(k == 7))
                    nc.scalar.activation(silu_z[m - 4][:, hs(h)], ps[:], AF.Silu)

            # ================= post-AR phase 4 for one half =================
            def phase4(h):
                xdt = sa.tile([64, LH], BF16, tag="xdt")
                nc.sync.dma_start(xdt[:], cc_out[h][0:64, :])
                xbc = sa.tile([32, LH], BF16, tag="xbc")
                nc.sync.dma_start(xbc[:], cc_out[h][64:96, :])
                for which, dest in ((0, b_rep), (1, c_rep)):
                    ps = pss.tile([128, LH], F32, tag="ps")
                    nc.tensor.matmul(ps[:], selbc_sb[:, which * 128:(which + 1) * 128],
                                     xbc[:], start=True, stop=True)
                    nc.scalar.copy(dest[:, hs(h)], ps[:])
                for q in range(NQ):
                    ps = pss.tile([128, LH], F32, tag="ps")
                    nc.tensor.matmul(ps[:], w_dt_sb[:, q * 128:(q + 1) * 128],
                                     xdt[:], start=True, stop=True)
                    spe = sp2.tile([128, LH], F32, tag="spe")
                    nc.scalar.activation(spe[:], ps[:], AF.Exp, bias=dtb_sb[:, q:q + 1])
                    nc.scalar.activation(delta[q][:, hs(h)], spe[:], AF.Ln, bias=1.0)
                    nc.vector.tensor_tensor(du[q][:, hs(h)], delta[q][:, hs(h)],
                                            u[q][:, hs(h)], op=OP.mult)
                    nc.gpsimd.dma_start(du_d[q][:, hs(h)], du[q][:, hs(h)])
                # prefetch du replication tiles (DRAM -> SBUF, gpsimd queue)
                dureps = {}
                for q in range(NQ):
                    for hf in range(2):
                        t = drp.tile([128, 8 * LH], BF16, tag="durep",
                                     name=f"durep{q}_{hf}_{h}")
                        srcv = du_d[q][hf * 64:(hf + 1) * 64, hs(h)].rearrange(
                            "(j d) l -> d j l", d=8)
                        for n in range(16):
                            nc.gpsimd.dma_start(
                                t[n * 8:(n + 1) * 8, :].rearrange(
                                    "d (j l) -> d j l", j=8),
                                srcv)
                        dureps[(q, hf)] = t
                return dureps

            # ============ phase 5 for one (half, hf, channel-pair) ============
            def phase5_unit(h, hf, qp, dureps, yps):
                areps = {}
                for q in qp:
                    areps[q] = gp.tile([128, 8 * LH], BF16, tag="arep",
                                       name=f"a{q}_{hf}_{h}")
                # selector matmuls j-outer so the stationary is loaded once per j
                for jj in range(8):
                    j = hf * 8 + jj
                    for q in qp:
                        g = q * 16 + j
                        psd = psr.tile([128, LH], F32, tag="ps_rep")
                        nc.tensor.matmul(psd[:], selr_sb[:, j * 128:(j + 1) * 128],
                                         delta[q][:, hs(h)], start=True, stop=True)
                        nc.scalar.activation(areps[q][:, jj * LH:(jj + 1) * LH],
                                             psd[:], AF.Exp, bias=0.0,
                                             scale=acol_sb[:, g:g + 1])
                for q in qp:
                    durep = dureps[(q, hf)]
                    # bu = du_rep * B (broadcast over group-blocks), in place
                    nc.vector.tensor_tensor(
                        durep[:].rearrange("p (j l) -> p j l", j=8),
                        durep[:].rearrange("p (j l) -> p j l", j=8),
                        b_rep[:, hs(h)].unsqueeze(1).broadcast_to([128, 8, LH]),
                        op=OP.mult)
                    arep = areps[q]
                    a3 = arep[:].rearrange("p (j l) -> p j l", l=LH)
                    bu3 = durep[:].rearrange("p (j l) -> p j l", l=LH)
                    cr = carry[q * 2 + hf]
                    if h == 0:
                        nc.vector.memset(a3[:, 1:8, 0:1], 0.0)
                    else:
                        tmp = sa.tile([128, 8], BF16, tag="tmp")
                        nc.vector.tensor_tensor(tmp[:].unsqueeze(2), a3[:, :, 0:1],
                                                cr[:].unsqueeze(2), op=OP.mult)
                        nc.vector.tensor_tensor(bu3[:, :, 0:1], bu3[:, :, 0:1],
                                                tmp[:].unsqueeze(2), op=OP.add)
                        nc.vector.memset(a3[:, :, 0:1], 0.0)
                    # scan in place: arep becomes h
                    nc.vector.tensor_tensor_scan(arep[:], arep[:], durep[:],
                                                 0.0, OP.mult, OP.add)
                    if h == 0:
                        nc.vector.tensor_copy(cr[:].unsqueeze(2), a3[:, :, LH - 1:LH])
                    # g = h * C in place: arep becomes g
                    nc.vector.tensor_tensor(
                        a3, a3,
                        c_rep[:, hs(h)].unsqueeze(1).broadcast_to([128, 8, LH]),
                        op=OP.mult)
                for jj in range(8):
                    j = hf * 8 + jj
                    for q in qp:
                        nc.tensor.matmul(yps[q][:], selo_sb[:, j * 128:(j + 1) * 128],
                                         areps[q][:, jj * LH:(jj + 1) * LH],
                                         start=(hf == 0 and jj == 0),
                                         stop=(hf == 1 and jj == 7),
                                         skip_group_check=True)

            def ygate_q(h, q, yps):
                t1 = sp2.tile([128, LH], F32, tag="t1")
                nc.vector.scalar_tensor_tensor(
                    t1[:], u[q][:, hs(h)], dcol_sb[:, q:q + 1], yps[:],
                    op0=OP.mult, op1=OP.add)
                nc.vector.tensor_tensor(ygate[q][:, hs(h)], t1[:],
                                        silu_z[q][:, hs(h)], op=OP.mult)

            def outproj(h):
                for m in range(8):
                    ps = pss.tile([128, LH], F32, tag="ps")
                    for q in range(NQ):
                        nc.tensor.matmul(
                            ps[:],
                            w_out_sb[:, q * 1024 + m * 128:q * 1024 + (m + 1) * 128],
                            ygate[q][:, hs(h)], start=(q == 0), stop=(q == 3))
                    ot = sp2.tile([128, LH], BF16, tag="ot")
                    nc.scalar.copy(ot[:], ps[:])
                    nc.sync.dma_start(out.ap()[m * 128:(m + 1) * 128, hs(h)], ot[:])

            # ================= schedule =================
            phases123(0)
            phases123(1)          # runs on PE while AR(0) is in flight
            for h in range(2):
                dureps = phase4(h)
                emitted_op0 = False
                for qp in ((0, 1), (2, 3)):
                    yps = {q: psy.tile([128, LH], F32, tag="ps_y",
                                       name=f"yps{q}_{h}") for q in qp}
                    for hf in range(2):
                        phase5_unit(h, hf, qp, dureps, yps)
                        if not emitted_op0 and h == 1:
                            emitted_op0 = True
                            outproj(0)
                    for q in qp:
                        ygate_q(h, q, yps[q])
            outproj(1)

    nc.compile()
    return nc


def _prep_core_inputs(c, x, in_proj_w, conv_w, conv_b, x_proj_w, dt_proj_w,
                      dt_proj_b, A_log, D, out_proj_w, sel_r, sel_o, sel_bc):
    b, s = divmod(c, 4)
    sl = slice(s * DL, (s + 1) * DL)
    bf = ml_dtypes.bfloat16
    A = (-np.exp(A_log[sl])).astype(np.float32)            # [512, 16]
    a_cols = np.empty((128, NGRP), np.float32)
    p = np.arange(128)
    for g in range(NGRP):
        a_cols[:, g] = A[g * 8 + (p % 8), p // 8]
    w_in_loc = np.concatenate([in_proj_w[sl], in_proj_w[2048 + s * DL:2048 + (s + 1) * DL]], 0)
    convd = np.zeros((128, NQ * 4 * 128), np.float32)
    cw = conv_w[sl, 0, :]                                  # [512, 4]
    for q in range(NQ):
        for k in range(4):
            blk = (q * 4 + k) * 128
            convd[np.arange(128), blk + np.arange(128)] = cw[q * 128:(q + 1) * 128, k]
    return {
        "xT": np.ascontiguousarray(x[b].T).astype(bf),
        "w_in": np.ascontiguousarray(w_in_loc.T).astype(bf),
        "w_xp": np.ascontiguousarray(x_proj_w[:, sl].T).astype(bf),
        "w_dt": np.ascontiguousarray(dt_proj_w[sl].T).astype(bf),
        "dt_b": np.ascontiguousarray(dt_proj_b[sl].reshape(NQ, 128).T).astype(np.float32),
        "w_out": np.ascontiguousarray(out_proj_w[:, sl].T).astype(bf),
        "a_cols": a_cols,
        "d_col": np.ascontiguousarray(D[sl].reshape(NQ, 128).T).astype(np.float32),
        "convd": convd.astype(bf),
        "convb": np.ascontiguousarray(conv_b[sl].reshape(NQ, 128).T).astype(np.float32),
        "selr": sel_r,
        "selo": sel_o,
        "selbc": sel_bc,
    }


def _selectors():
    bf = ml_dtypes.bfloat16
    p = np.arange(128)
    sel_r = np.zeros((128, 16 * 128), dtype=bf)
    sel_o = np.zeros((128, 16 * 128), dtype=bf)
    for j in range(16):
        sel_r[j * 8 + (p % 8), j * 128 + p] = 1       # replicate 8 ch -> (n,d)
        sel_o[p, j * 128 + j * 8 + (p % 8)] = 1       # reduce states back
    sel_bc = np.zeros((32, 2 * 128), dtype=bf)
    sel_bc[p // 8, p] = 1                              # B: rows 0:16 -> n-major
    sel_bc[16 + p // 8, 128 + p] = 1                   # C: rows 16:32
    return sel_r, sel_o, sel_bc


def kernel(x, in_proj_w, conv_w, conv_b, x_proj_w, dt_proj_w, dt_proj_b,
           A_log, D, out_proj_w):
    sel_r, sel_o, sel_bc = _selectors()
    if "nc" not in _CACHE:
        _CACHE["nc"] = _build()
    nc = _CACHE["nc"]

    args = (x, in_proj_w, conv_w, conv_b, x_proj_w, dt_proj_w, dt_proj_b,
            A_log, D, out_proj_w)
    in_maps = [_prep_core_inputs(c, *args, sel_r, sel_o, sel_bc) for c in range(8)]
    res = bass_utils.run_bass_kernel_spmd(nc, in_maps, core_ids=list(range(8)))
    outs = res.results
    _CACHE["last_result"] = res

    full = np.zeros((2, L, DM), dtype=np.float32)
    for b in range(2):
        acc = outs[4 * b]["out"].astype(np.float32)
        for s in range(1, 4):
            acc = acc + outs[4 * b + s]["out"].astype(np.float32)
        full[b] = acc.T
    return full


# revision 12
# speedup vs baseline: 1.0694x; 1.0694x over previous
"""Mamba-1 selective SSM block on 8 trn2 NeuronCores.

Sharding: 2 batch-groups x 4 channel-shards. Core c handles batch c//4 and
d_inner channels [(c%4)*512, (c%4+1)*512). Cross-core comm: bf16 AllReduce of
the x_proj partial [96, 512] per L-half within each 4-core batch group. Host
sums the 4 partial out_proj outputs per batch.

The network is software-pipelined over two L-halves so the AllReduce hides
under compute of the other half. State tensors live as tiles of
[128 partitions, 8*LH] where partition p = n*8 + d_sub covers 8 channels x 16
states and the free dim concatenates 8 groups' L-half segments; one DVE scan
per (q, hf, half) covers 8 groups using a=0 segment resets (the carry across
halves is folded into the first bu element of each segment). delta replicates
onto (n,d) via PE selector matmuls + Act exp; du replicates via DRAM DMA
issued from the idle gpsimd queue. Selector matmuls run j-outer over channel
pairs so LDWEIGHTS is shared.
"""

import numpy as np
import ml_dtypes

import concourse.bacc as bacc
import concourse.mybir as mybir
import concourse.tile as tile
from concourse import bass_utils

BF16 = mybir.dt.bfloat16
F32 = mybir.dt.float32
AF = mybir.ActivationFunctionType
OP = mybir.AluOpType

L = 1024          # sequence length
DM = 1024         # model dim
DL = 512          # local d_inner channels per core
NQ = 4            # channel chunks of 128 per core
NGRP = 64         # DL/8 groups per core
RANK = 64         # dt_rank
LH = 512          # L half

_CACHE = {}


def _build(sim=False):
    nc = bacc.Bacc("TRN2", target_bir_lowering=False, debug=False, num_devices=8)

    xT = nc.dram_tensor("xT", [DM, L], BF16, kind="ExternalInput")
    w_in = nc.dram_tensor("w_in", [DM, 2 * DL], BF16, kind="ExternalInput")
    w_xp = nc.dram_tensor("w_xp", [DL, 96], BF16, kind="ExternalInput")
    w_dt = nc.dram_tensor("w_dt", [RANK, DL], BF16, kind="ExternalInput")
    dt_b = nc.dram_tensor("dt_b", [128, NQ], F32, kind="ExternalInput")
    w_out = nc.dram_tensor("w_out", [DL, DM], BF16, kind="ExternalInput")
    a_cols = nc.dram_tensor("a_cols", [128, NGRP], F32, kind="ExternalInput")
    d_col = nc.dram_tensor("d_col", [128, NQ], F32, kind="ExternalInput")
    convd = nc.dram_tensor("convd", [128, NQ * 4 * 128], BF16, kind="ExternalInput")
    convb = nc.dram_tensor("convb", [128, NQ], F32, kind="ExternalInput")
    selr = nc.dram_tensor("selr", [128, 16 * 128], BF16, kind="ExternalInput")
    selo = nc.dram_tensor("selo", [128, 16 * 128], BF16, kind="ExternalInput")
    selbc = nc.dram_tensor("selbc", [32, 2 * 128], BF16, kind="ExternalInput")
    out = nc.dram_tensor("out", [DM, L], BF16, kind="ExternalOutput")

    with tile.TileContext(nc) as tc:
        with (
            tc.tile_pool(name="const", bufs=1) as cp,
            tc.tile_pool(name="acts", bufs=1) as ap,
            tc.tile_pool(name="wpool", bufs=1) as wp,
            tc.tile_pool(name="dram", bufs=1, space="DRAM") as dp,
            tc.tile_pool(name="grp", bufs=4) as gp,
            tc.tile_pool(name="small", bufs=2) as sp2,
            tc.tile_pool(name="sa", bufs=2) as sa,
            tc.tile_pool(name="ps_small", bufs=2, space="PSUM") as pss,
            tc.tile_pool(name="ps_rep", bufs=4, space="PSUM") as psr,
            tc.tile_pool(name="ps_y", bufs=2, space="PSUM") as psy,
        ):
            # ---- persistent weights (critical-path order) ----
            xT_sb = wp.tile([128, 8 * L], BF16, tag="xT")
            nc.sync.dma_start(xT_sb[:].rearrange("p (k l) -> p k l", k=8),
                              xT.ap().rearrange("(k p) l -> p k l", p=128))
            w_in_sb = wp.tile([128, 8 * 1024], BF16, tag="w_in")
            nc.sync.dma_start(w_in_sb[:].rearrange("p (k m) -> p k m", k=8),
                              w_in.ap().rearrange("(k p) m -> p k m", p=128))
            convd_sb = cp.tile([128, NQ * 4 * 128], BF16, tag="convd")
            nc.sync.dma_start(convd_sb[:], convd.ap())
            convb_sb = cp.tile([128, NQ], F32, tag="convb")
            nc.sync.dma_start(convb_sb[:], convb.ap())
            w_xp_sb = wp.tile([128, 4 * 96], BF16, tag="w_xp")
            nc.sync.dma_start(w_xp_sb[:].rearrange("p (k m) -> p k m", k=4),
                              w_xp.ap().rearrange("(k p) m -> p k m", p=128))
            w_dt_sb = wp.tile([64, DL], BF16, tag="w_dt")
            nc.sync.dma_start(w_dt_sb[:], w_dt.ap())
            selr_sb = cp.tile([128, 16 * 128], BF16, tag="selr")
            nc.sync.dma_start(selr_sb[:], selr.ap())
            selo_sb = cp.tile([128, 16 * 128], BF16, tag="selo")
            nc.sync.dma_start(selo_sb[:], selo.ap())
            selbc_sb = cp.tile([32, 2 * 128], BF16, tag="selbc")
            nc.sync.dma_start(selbc_sb[:], selbc.ap())
            dtb_sb = cp.tile([128, NQ], F32, tag="dtb")
            nc.sync.dma_start(dtb_sb[:], dt_b.ap())
            acol_sb = cp.tile([128, NGRP], F32, tag="acol")
            nc.sync.dma_start(acol_sb[:], a_cols.ap())
            dcol_sb = cp.tile([128, NQ], F32, tag="dcol")
            nc.sync.dma_start(dcol_sb[:], d_col.ap())
            w_out_sb = wp.tile([128, 4 * 1024], BF16, tag="w_out")
            nc.sync.dma_start(w_out_sb[:].rearrange("p (k m) -> p k m", k=4),
                              w_out.ap().rearrange("(k p) m -> p k m", p=128))

            # ---- persistent activations ----
            xin = [ap.tile([128, 3 + L], BF16, tag=f"xin{q}", name=f"xin{q}") for q in range(NQ)]
            silu_z = [ap.tile([128, L], BF16, tag=f"sz{q}", name=f"sz{q}") for q in range(NQ)]
            u = [ap.tile([128, L], BF16, tag=f"u{q}", name=f"u{q}") for q in range(NQ)]
            delta = [ap.tile([128, L], BF16, tag=f"delta{q}", name=f"delta{q}") for q in range(NQ)]
            du = [ap.tile([128, L], BF16, tag=f"du{q}", name=f"du{q}") for q in range(NQ)]
            ygate = [ap.tile([128, L], BF16, tag=f"yg{q}", name=f"yg{q}") for q in range(NQ)]
            b_rep = ap.tile([128, L], BF16, tag="brep")
            c_rep = ap.tile([128, L], BF16, tag="crep")
            carry = [ap.tile([128, 8], BF16, tag=f"carry{i}", name=f"carry{i}")
                     for i in range(2 * NQ)]
            cc_in = [dp.tile([96, LH], BF16, name=f"cc_in{h}") for h in range(2)]
            cc_out = [dp.tile([96, LH], BF16, name=f"cc_out{h}") for h in range(2)]

            for q in range(NQ):
                nc.vector.memset(xin[q][:, 0:3], 0.0)

            def hs(h):
                return slice(h * LH, (h + 1) * LH)

            def xs(h):
                return slice(3 + h * LH, 3 + (h + 1) * LH)

            # ================= phases 1-3 for one half =================
            def phases123(h):
                for q in range(NQ):
                    ps = pss.tile([128, LH], F32, tag="ps")
                    for k in range(8):
                        nc.tensor.matmul(
                            ps[:],
                            w_in_sb[:, q * 128 + k * 1024:(q + 1) * 128 + k * 1024],
                            xT_sb[:, k * L + h * LH:k * L + h * LH + LH],
                            start=(k == 0), stop=(k == 7))
                    nc.scalar.copy(xin[q][:, xs(h)], ps[:])
                    psc = pss.tile([128, LH], F32, tag="ps")
                    for k in range(4):
                        nc.tensor.matmul(
                            psc[:],
                            convd_sb[:, (q * 4 + k) * 128:(q * 4 + k + 1) * 128],
                            xin[q][:, k + h * LH:k + h * LH + LH],
                            start=(k == 0), stop=(k == 3))
                    nc.scalar.activation(u[q][:, hs(h)], psc[:], AF.Silu,
                                         bias=convb_sb[:, q:q + 1])
                # x_proj partial -> DRAM (bf16) -> AllReduce
                psx = pss.tile([96, LH], F32, tag="ps")
                for q in range(NQ):
                    nc.tensor.matmul(psx[:], w_xp_sb[:, q * 96:(q + 1) * 96],
                                     u[q][:, hs(h)], start=(q == 0), stop=(q == 3))
                xpc = sa.tile([96, LH], BF16, tag="xpc")
                nc.scalar.copy(xpc[:], psx[:])
                nc.sync.dma_start(cc_in[h][:], xpc[:])
                if sim:
                    nc.sync.dma_start(cc_out[h][:], cc_in[h][:])
                else:
                    nc.gpsimd.collective_compute(
                        "AllReduce", OP.add,
                        replica_groups=[[0, 1, 2, 3], [4, 5, 6, 7]],
                        ins=[cc_in[h].opt()], outs=[cc_out[h].opt()])
                # z half (not needed until ygate; runs during AR)
                for m in range(4, 8):
                    ps = pss.tile([128, LH], F32, tag="ps")
                    for k in range(8):
                        nc.tensor.matmul(
                            ps[:],
                            w_in_sb[:, m * 128 + k * 1024:(m + 1) * 128 + k * 1024],
                            xT_sb[:, k * L + h * LH:k * L + h * LH + LH],
                            start=(k == 0), stop=(k == 7))
                    nc.scalar.activation(silu_z[m - 4][:, hs(h)], ps[:], AF.Silu)

            # ================= post-AR phase 4 for one half =================
            def phase4(h):
                xdt = sa.tile([64, LH], BF16, tag="xdt")
                nc.sync.dma_start(xdt[:], cc_out[h][0:64, :])
                xbc = sa.tile([32, LH], BF16, tag="xbc")
                nc.sync.dma_start(xbc[:], cc_out[h][64:96, :])
                for which, dest in ((0, b_rep), (1, c_rep)):
                    ps = pss.tile([128, LH], F32, tag="ps")
                    nc.tensor.matmul(ps[:], selbc_sb[:, which * 128:(which + 1) * 128],
                                     xbc[:], start=True, stop=True)
                    nc.scalar.copy(dest[:, hs(h)], ps[:])
                for q in range(NQ):
                    ps = pss.tile([128, LH], F32, tag="ps")
                    nc.tensor.matmul(ps[:], w_dt_sb[:, q * 128:(q + 1) * 128],
                                     xdt[:], start=True, stop=True)
                    spe = sp2.tile([128, LH], F32, tag="spe")
                    nc.scalar.activation(spe[:], ps[:], AF.Exp, bias=dtb_sb[:, q:q + 1])
                    nc.scalar.activation(delta[q][:, hs(h)], spe[:], AF.Ln, bias=1.0)
                    nc.vector.tensor_tensor(du[q][:, hs(h)], delta[q][:, hs(h)],
                                            u[q][:, hs(h)], op=OP.mult)

            # ============ phase 5 for one (half, hf, channel-pair) ============
            def phase5_unit(h, hf, qp, yps):
                areps = {}
                dureps = {}
                for q in qp:
                    areps[q] = gp.tile([128, 8 * LH], BF16, tag="arep",
                                       name=f"a{q}_{hf}_{h}")
                    dureps[q] = gp.tile([128, 8 * LH], BF16, tag="durep",
                                        name=f"d{q}_{hf}_{h}")
                # selector matmuls j-outer so the stationary is loaded once per
                # j; du replication rides the same stationary and evicts via an
                # Act copy (same act table as Exp)
                for jj in range(8):
                    j = hf * 8 + jj
                    for q in qp:
                        g = q * 16 + j
                        psd = psr.tile([128, LH], F32, tag="ps_rep")
                        nc.tensor.matmul(psd[:], selr_sb[:, j * 128:(j + 1) * 128],
                                         delta[q][:, hs(h)], start=True, stop=True)
                        psd2 = psr.tile([128, LH], F32, tag="ps_rep")
                        nc.tensor.matmul(psd2[:], selr_sb[:, j * 128:(j + 1) * 128],
                                         du[q][:, hs(h)], start=True, stop=True)
                        nc.scalar.activation(areps[q][:, jj * LH:(jj + 1) * LH],
                                             psd[:], AF.Exp, bias=0.0,
                                             scale=acol_sb[:, g:g + 1])
                        nc.scalar.copy(dureps[q][:, jj * LH:(jj + 1) * LH], psd2[:])
                for q in qp:
                    durep = dureps[q]
                    # bu = du_rep * B (broadcast over group-blocks), in place
                    nc.vector.tensor_tensor(
                        durep[:].rearrange("p (j l) -> p j l", j=8),
                        durep[:].rearrange("p (j l) -> p j l", j=8),
                        b_rep[:, hs(h)].unsqueeze(1).broadcast_to([128, 8, LH]),
                        op=OP.mult)
                    arep = areps[q]
                    a3 = arep[:].rearrange("p (j l) -> p j l", l=LH)
                    bu3 = durep[:].rearrange("p (j l) -> p j l", l=LH)
                    cr = carry[q * 2 + hf]
                    if h == 0:
                        nc.vector.memset(a3[:, 1:8, 0:1], 0.0)
                    else:
                        tmp = sa.tile([128, 8], BF16, tag="tmp")
                        nc.vector.tensor_tensor(tmp[:].unsqueeze(2), a3[:, :, 0:1],
                                                cr[:].unsqueeze(2), op=OP.mult)
                        nc.vector.tensor_tensor(bu3[:, :, 0:1], bu3[:, :, 0:1],
                                                tmp[:].unsqueeze(2), op=OP.add)
                        nc.vector.memset(a3[:, :, 0:1], 0.0)
                    # scan in place: arep becomes h
                    nc.vector.tensor_tensor_scan(arep[:], arep[:], durep[:],
                                                 0.0, OP.mult, OP.add)
                    if h == 0:
                        nc.vector.tensor_copy(cr[:].unsqueeze(2), a3[:, :, LH - 1:LH])
                    # g = h * C in place: arep becomes g
                    nc.vector.tensor_tensor(
                        a3, a3,
                        c_rep[:, hs(h)].unsqueeze(1).broadcast_to([128, 8, LH]),
                        op=OP.mult)
                for jj in range(8):
                    j = hf * 8 + jj
                    for q in qp:
                        nc.tensor.matmul(yps[q][:], selo_sb[:, j * 128:(j + 1) * 128],
                                         areps[q][:, jj * LH:(jj + 1) * LH],
                                         start=(hf == 0 and jj == 0),
                                         stop=(hf == 1 and jj == 7),
                                         skip_group_check=True)

            def ygate_q(h, q, yps):
                t1 = sp2.tile([128, LH], F32, tag="t1")
                nc.vector.scalar_tensor_tensor(
                    t1[:], u[q][:, hs(h)], dcol_sb[:, q:q + 1], yps[:],
                    op0=OP.mult, op1=OP.add)
                nc.vector.tensor_tensor(ygate[q][:, hs(h)], t1[:],
                                        silu_z[q][:, hs(h)], op=OP.mult)

            def outproj(h):
                for m in range(8):
                    ps = pss.tile([128, LH], F32, tag="ps")
                    for q in range(NQ):
                        nc.tensor.matmul(
                            ps[:],
                            w_out_sb[:, q * 1024 + m * 128:q * 1024 + (m + 1) * 128],
                            ygate[q][:, hs(h)], start=(q == 0), stop=(q == 3))
                    ot = sp2.tile([128, LH], BF16, tag="ot")
                    nc.scalar.copy(ot[:], ps[:])
                    nc.sync.dma_start(out.ap()[m * 128:(m + 1) * 128, hs(h)], ot[:])

            # ================= schedule =================
            phases123(0)
            phases123(1)          # runs on PE while AR(0) is in flight
            for h in range(2):
                phase4(h)
                emitted_op0 = False
                for qp in ((0, 1), (2, 3)):
                    yps = {q: psy.tile([128, LH], F32, tag="ps_y",
                                       name=f"yps{q}_{h}") for q in qp}
                    for hf in range(2):
                        phase5_unit(h, hf, qp, yps)
                        if not emitted_op0 and h == 1:
                            emitted_op0 = True
                            outproj(0)
                    for q in qp:
                        ygate_q(h, q, yps[q])
            outproj(1)

    nc.compile()
    return nc


def _prep_core_inputs(c, x, in_proj_w, conv_w, conv_b, x_proj_w, dt_proj_w,
                      dt_proj_b, A_log, D, out_proj_w, sel_r, sel_o, sel_bc):
    b, s = divmod(c, 4)
    sl = slice(s * DL, (s + 1) * DL)
    bf = ml_dtypes.bfloat16
    A = (-np.exp(A_log[sl])).astype(np.float32)            # [512, 16]
    a_cols = np.empty((128, NGRP), np.float32)
    p = np.arange(128)
    for g in range(NGRP):
        a_cols[:, g] = A[g * 8 + (p % 8), p // 8]
    w_in_loc = np.concatenate([in_proj_w[sl], in_proj_w[2048 + s * DL:2048 + (s + 1) * DL]], 0)
    convd = np.zeros((128, NQ * 4 * 128), np.float32)
    cw = conv_w[sl, 0, :]                                  # [512, 4]
    for q in range(NQ):
        for k in range(4):
            blk = (q * 4 + k) * 128
            convd[np.arange(128), blk + np.arange(128)] = cw[q * 128:(q + 1) * 128, k]
    return {
        "xT": np.ascontiguousarray(x[b].T).astype(bf),
        "w_in": np.ascontiguousarray(w_in_loc.T).astype(bf),
        "w_xp": np.ascontiguousarray(x_proj_w[:, sl].T).astype(bf),
        "w_dt": np.ascontiguousarray(dt_proj_w[sl].T).astype(bf),
        "dt_b": np.ascontiguousarray(dt_proj_b[sl].reshape(NQ, 128).T).astype(np.float32),
        "w_out": np.ascontiguousarray(out_proj_w[:, sl].T).astype(bf),
        "a_cols": a_cols,
        "d_col": np.ascontiguousarray(D[sl].reshape(NQ, 128).T).astype(np.float32),
        "convd": convd.astype(bf),
        "convb": np.ascontiguousarray(conv_b[sl].reshape(NQ, 128).T).astype(np.float32),
        "selr": sel_r,
        "selo": sel_o,
        "selbc": sel_bc,
    }


def _selectors():
    bf = ml_dtypes.bfloat16
    p = np.arange(128)
    sel_r = np.zeros((128, 16 * 128), dtype=bf)
    sel_o = np.zeros((128, 16 * 128), dtype=bf)
    for j in range(16):
        sel_r[j * 8 + (p % 8), j * 128 + p] = 1       # replicate 8 ch -> (n,d)
        sel_o[p, j * 128 + j * 8 + (p % 8)] = 1       # reduce states back
    sel_bc = np.zeros((32, 2 * 128), dtype=bf)
    sel_bc[p // 8, p] = 1                              # B: rows 0:16 -> n-major
    sel_bc[16 + p // 8, 128 + p] = 1                   # C: rows 16:32
    return sel_r, sel_o, sel_bc


def kernel(x, in_proj_w, conv_w, conv_b, x_proj_w, dt_proj_w, dt_proj_b,
           A_log, D, out_proj_w):
    sel_r, sel_o, sel_bc = _selectors()
    if "nc" not in _CACHE:
        _CACHE["nc"] = _build()
    nc = _CACHE["nc"]

    args = (x, in_proj_w, conv_w, conv_b, x_proj_w, dt_proj_w, dt_proj_b,
            A_log, D, out_proj_w)
    in_maps = [_prep_core_inputs(c, *args, sel_r, sel_o, sel_bc) for c in range(8)]
    res = bass_utils.run_bass_kernel_spmd(nc, in_maps, core_ids=list(range(8)))
    outs = res.results
    _CACHE["last_result"] = res

    full = np.zeros((2, L, DM), dtype=np.float32)
    for b in range(2):
        acc = outs[4 * b]["out"].astype(np.float32)
        for s in range(1, 4):
            acc = acc + outs[4 * b + s]["out"].astype(np.float32)
        full[b] = acc.T
    return full
